# revision 21
# baseline (speedup 1.0000x reference)
"""Viterbi CRF decode on Trainium2 (Bass), 8-core data-parallel.

Problem: B=128, S=512, T=32 (30 labels + START=30, END=31).
  forward max-plus scan over S steps, backpointers, masked lengths,
  backward pointer-following pass. Output [B, S] int32 tag path.

Sharding: pure data parallel, 16 examples per core.

Per-core layout (SBUF partitions p = 32*q + j, quadrant q in [0,4) holds
examples b = 4q+br, br in [0,4); j in [0,32) is the tag index):
  - state P4[p, (br,i)] = part[b, i] (part vector replicated across the 32
    j-partitions of each quadrant)
  - forward step (5 DVE ops, fused dual reduce):
      RB[t%2] = [V_t (128) | W_{t-1} (128)] where V = scores + P4 and
      W = (V == part bcast) * (31 - i).
      RED2_t: one 8-segment max-reduce over RB[t%2] emits part_t AND the
      encoded backpointer row bpw_{t-1} into the interleaved history
      PHBW[p, 8t:8t+8] = [part_t (4), bpw_{t-1} (4)].
      eq_t, transpose_t (rebuild P4), mul_t, add_{t+1} fill the rest; the
      op order keeps >=1 op between every stream-shuffle write/read and
      the ALU ops that touch the same tensor (no HW write->read interlock).
  - pointer phase: arithmetic select of part at last valid position
    (monotone mask -> at-last indicator), argmax into END tag.
  - backward: per step, block-transpose of the bp row + one fused
    scalar_tensor_tensor (one-hot select, sum-accumulate) = the gather.

All compute on the vector engine (exact fp32, same association order as
the jax reference: (feats + trans) + part), DMA on sync engine.

The reps parameter is a HARDWARE loop bound (sequencer branch + register
counter): program size is independent of reps, so a reps=K vs reps=1
wall-clock difference isolates true device execution time of (K-1) body
passes instead of host-side per-instruction compile/serialize costs.
"""

import numpy as np
from contextlib import ExitStack

import concourse.bass as bass
import concourse.mybir as mybir
from concourse.bass_utils import run_bass_kernel_spmd

F32 = mybir.dt.float32
I32 = mybir.dt.int32
AX = mybir.AxisListType
OP = mybir.AluOpType

T = 32
START = 30
END = 31
NCORES = 8


def build_nc(S, reps=1, sim_compat=False):
    # Single compute engine (DVE) in program order: same-engine RAW/WAW is
    # serialized by the hardware (per-op pipe drain); the conservative race
    # detector does not model engine ordering, so it is disabled.
    nc = bass.Bass(detect_race_conditions=False)
    ft_d = nc.declare_dram_parameter("ft", [128, 4 * S], F32, isOutput=False)
    mkf_d = nc.declare_dram_parameter("mkf", [128, 4 * S + 4], F32, isOutput=False)
    tt_d = nc.declare_dram_parameter("tt", [128, 32], F32, isOutput=False)
    cst_d = nc.declare_dram_parameter("cst", [128, 64], F32, isOutput=False)
    ct2_d = nc.declare_dram_parameter("ct2", [128, 128], F32, isOutput=False)
    dec_d = nc.declare_dram_parameter("dec", [128, S], I32, isOutput=True)

    K = S - 1  # bp rows k in [0, K)

    with ExitStack() as ctx:
        def sb(name, shape, dt=F32):
            return ctx.enter_context(nc.sbuf_tensor(name, shape, dt))

        FT = sb("FT", [128, 4 * S])
        MKF = sb("MKF", [128, 4 * S + 4])
        TT = sb("TT", [128, 32])
        # interleaved history: [part_t (4) | bpw_{t-1} (4)] at cols 8t
        PHBW = sb("PHBW", [128, 8 * S + 16])
        RB0 = sb("RB0", [128, 256])  # [V | W] ping
        RB1 = sb("RB1", [128, 256])  # [V | W] pong
        EB = sb("EB", [128, 128])
        XS = sb("XS", [128, 4 * S + 32])
        XS2 = sb("XS2", [128, 4 * S + 32])
        SCR = sb("SCR", [128, 4 * K])
        ALF = sb("ALF", [128, 4 * S])
        ALB = sb("ALB", [128, 4 * S])
        SCH = sb("SCH", [128, 64 * 128])
        SCHB = sb("SCHB", [128, 64 * 128])
        CT2 = sb("CT2", [128, 128])
        P4 = sb("P4", [128, 128])
        T32 = sb("T32", [128, 32])
        DEC = sb("DEC", [128, S])
        DECI = sb("DECI", [128, S], I32)
        CST = sb("CST", [128, 64])
        TB32 = sb("TB32", [128, 1024])  # 32-slot ring of transposed bp rows
        TEND = sb("TEND", [128, 32])
        LPP = sb("LPP", [128, 32])
        TLP = sb("TLP", [128, 32])
        CAND = sb("CAND", [128, 32])
        MX = sb("MX", [128, 1])
        EQC = sb("EQC", [128, 32])
        PW = sb("PW", [128, 1])
        P32 = sb("P32", [128, 32])
        PR = sb("PR", [128, 32])
        SC = sb("SC", [128, 32])

        RB = [RB0, RB1]

        with (
            nc.semaphore() as dma_sem,
            nc.semaphore() as done_sem,
            nc.semaphore() as sch_sem,
            nc.semaphore() as cons_sem,
            nc.Block() as block,
        ):
            @block.sync
            def _(sync):
                sync.dma_start(out=FT[:], in_=ft_d[:]).then_inc(dma_sem, 16)
                sync.dma_start(out=MKF[:], in_=mkf_d[:]).then_inc(dma_sem, 16)
                sync.dma_start(out=TT[:], in_=tt_d[:]).then_inc(dma_sem, 16)
                sync.dma_start(out=CST[:], in_=cst_d[:]).then_inc(dma_sem, 16)
                sync.dma_start(out=CT2[:], in_=ct2_d[:]).then_inc(dma_sem, 16)
                sync.wait_ge(done_sem, 1)
                sync.dma_start(out=dec_d[:], in_=DECI[:]).then_inc(dma_sem, 16)

            # score-chunk producer: chunks 1..7 built on the (otherwise idle)
            # GPSIMD engine into a double buffer, paced by DVE consumption.
            # Register-held thresholds keep the monotonic semaphores correct
            # across reps-loop passes.
            tt_c = TT[:].unsqueeze(1).unsqueeze(1).broadcast_to([128, 64, 4, 32])

            def sch_src(c):
                return FT[:, 256 * c:256 * (c + 1)].rearrange(
                    "p (u b) -> p u b", b=4).unsqueeze(3).broadcast_to([128, 64, 4, 32])

            @block.gpsimd
            def _(g):
                g.wait_ge(dma_sem, 80)
                with g.register("rp") as rp:
                    g.reg_mov(rp, 0)
                    with g.Fori(0, reps):
                        # chunk c reuses the buffer of chunk c-2 (or, for
                        # c=1, of chunk 7 from the previous pass); DVE incs
                        # cons_sem once per consumed chunk in {0..5, 7}, so
                        # a wait at rp, rp+=1 per chunk lines up exactly.
                        for c in range(1, 8):
                            g.wait_ge(cons_sem, rp)
                            g.reg_add(rp, rp, 1)
                            buf = SCH if c % 2 == 0 else SCHB
                            g.tensor_tensor(
                                out=buf[:].rearrange("p (u b i) -> p u b i", b=4, i=32),
                                in0=sch_src(c), in1=tt_c, op=OP.add,
                            ).then_inc(sch_sem, 1)

            def emit_body(v, rs):
                # ---- constants / scratch init ----
                v.stream_shuffle(out=TEND[:], in_=TT[:], mask=[END] * 32)
                v.memset(XS[:, 4 * K:], 0.0)
                v.memset(RB0[:, 128:256], 0.0)
                v.memset(RB1[:, 128:256], 0.0)
                v.memset(P32[:], 0.0)
                v.memset(LPP[:], 0.0)

                # init t=0: part0[b, j] = feats[b,0,j] + trans[START, j]
                v.tensor_scalar_add(out=PHBW[:, 0:4], in0=FT[:, 0:4],
                                    scalar1=TT[:, START:START + 1])
                # independent fillers keep an op of distance between
                # end-of-stream writes and start-of-stream shuffle reads
                v.tensor_sub(out=ALF[:], in0=MKF[:, 0:4 * S], in1=MKF[:, 4:4 * S + 4])
                # chunk 0 of the scores is built on DVE (needed immediately;
                # the GPSIMD producer covers chunks 1..7)
                v.tensor_tensor(out=SCH[:].rearrange("p (u b i) -> p u b i", b=4, i=32),
                                in0=sch_src(0), in1=tt_c, op=OP.add)
                p4_blk = P4[:].rearrange("p (b i) -> p b i", b=4)

                def p4_build(t0):
                    # replicate part_t0[b,:] to every partition of its quadrant:
                    # one 4-block stream-transpose with 0-stride input columns
                    if sim_compat:
                        for br in range(4):
                            v.transpose(out=P4[:, 32 * br:32 * br + 32],
                                        in_=PHBW[:, 8 * t0 + br:8 * t0 + br + 1].broadcast_to([128, 32]))
                    else:
                        v.transpose(out=p4_blk,
                                    in_=PHBW[:, 8 * t0:8 * t0 + 4].unsqueeze(2).broadcast_to([128, 4, 32]))

                p4_build(0)
                v.tensor_scalar(out=ALB[:], in0=ALF[:], scalar1=1.0,
                                scalar2=1e30, op0=OP.subtract, op1=OP.mult)
                # pre-loop: V_1 = scores_1 + P4_0
                v.tensor_tensor(out=RB1[:, 0:128], in0=SCH[:, 128:256],
                                in1=P4[:], op=OP.add)

                eb_v = EB[:].rearrange("p (b i) -> p b i", b=4)

                # ---- forward scan: 5 ops per step ----
                for t in range(1, S - 1):
                    rb_c = RB[t % 2]
                    rb_n = RB[(t + 1) % 2]
                    v.tensor_reduce(out=PHBW[:, 8 * t:8 * t + 8],
                                    in_=rb_c[:].rearrange("p (s i) -> p s i", s=8),
                                    axis=AX.X, op=OP.max)
                    v.tensor_tensor(out=eb_v,
                                    in0=rb_c[:, 0:128].rearrange("p (b i) -> p b i", b=4),
                                    in1=PHBW[:, 8 * t:8 * t + 4].unsqueeze(2).broadcast_to([128, 4, 32]),
                                    op=OP.is_equal)
                    p4_build(t)
                    v.tensor_tensor(out=rb_n[:, 128:256],
                                    in0=EB[:], in1=CT2[:], op=OP.mult)
                    u1 = (t + 1) % 64
                    c1 = (t + 1) // 64
                    if u1 == 0:
                        # new chunk: wait for the GPSIMD producer
                        v.reg_add(rs, rs, 1)
                        v.wait_ge(sch_sem, rs)
                    buf = SCH if c1 % 2 == 0 else SCHB
                    ins = v.tensor_tensor(out=rb_n[:, 0:128],
                                          in0=buf[:, 128 * u1:128 * u1 + 128],
                                          in1=P4[:], op=OP.add)
                    if u1 == 63 and c1 != 6:
                        # chunk c1 fully consumed; free its buffer (chunk 6's
                        # buffer is next written by DVE itself, no sem needed)
                        ins.then_inc(cons_sem, 1)

                # tail t = S-1: last partition + last bp row
                tl = S - 1
                v.tensor_reduce(out=PHBW[:, 8 * tl:8 * tl + 8],
                                in_=RB[tl % 2][:].rearrange("p (s i) -> p s i", s=8),
                                axis=AX.X, op=OP.max)
                v.tensor_tensor(out=eb_v,
                                in0=RB[tl % 2][:, 0:128].rearrange("p (b i) -> p b i", b=4),
                                in1=PHBW[:, 8 * tl:8 * tl + 4].unsqueeze(2).broadcast_to([128, 4, 32]),
                                op=OP.is_equal)
                v.tensor_tensor(out=RB[S % 2][:, 128:256],
                                in0=EB[:], in1=CT2[:], op=OP.mult)
                v.tensor_reduce(out=PHBW[:, 8 * S + 4:8 * S + 8],
                                in_=RB[S % 2][:, 128:256].rearrange("p (b i) -> p b i", b=4),
                                axis=AX.X, op=OP.max)

                # ---- last_partition by-i-partition: max over t of PH + ALB ----
                ph_tb = PHBW[:, 0:8 * S].rearrange("p (t c) -> p t c", c=8)[:, :, 0:4]
                alb_tb = ALB[:].rearrange("p (t b) -> p t b", b=4)
                xs_tb = XS[:, 0:4 * S].rearrange("p (t b) -> p t b", b=4)
                v.tensor_tensor(out=xs_tb, in0=ph_tb, in1=alb_tb, op=OP.add)
                v.tensor_reduce(out=LPP[:, 0:4],
                                in_=XS[:, 0:4 * S].rearrange("p (t b) -> p b t", b=4),
                                axis=AX.X, op=OP.max)

                # bp decode + mask (independent of LPP; also serves as filler)
                # bp row k = bpw_{k+1} at PHBW cols 8k+20 .. 8k+24
                bp_src = PHBW[:, 16:16 + 8 * K].rearrange("p (k c) -> p k c", c=8)[:, :, 4:8]
                xs2_kb = XS2[:, 0:4 * K].rearrange("p (k b) -> p k b", b=4)
                scr_kb = SCR[:].rearrange("p (k b) -> p k b", b=4)
                mkf_kb = MKF[:, 4:4 * K + 4].rearrange("p (k b) -> p k b", b=4)
                v.tensor_scalar(out=xs2_kb, in0=bp_src,
                                scalar1=-1.0, scalar2=31.0, op0=OP.mult, op1=OP.add)

                # pointer = argmax_i(LP[b,i] + trans[i,END]); one-time tail.
                # Independent ops (incl. the relocated bp-mask bulk op) serve
                # as spacers between each end-write -> start-read pair --
                # explicit drains cost ~2us each on HW.
                v.transpose(out=TLP[:], in_=LPP[:])
                v.tensor_tensor(out=scr_kb, in0=xs2_kb, in1=mkf_kb, op=OP.mult)
                v.tensor_tensor(out=CAND[:], in0=TLP[:], in1=TEND[:], op=OP.add)
                v.tensor_reduce(out=MX[:], in_=CAND[:], axis=AX.X, op=OP.max)
                v.tensor_copy(out=PR[:], in_=CST[:, 0:32])
                v.tensor_tensor(out=EQC[:], in0=CAND[:],
                                in1=MX[:].broadcast_to([128, 32]), op=OP.is_equal)
                v.tensor_tensor(out=SC[:], in0=EQC[:], in1=CST[:, 32:64], op=OP.mult)
                v.tensor_reduce(out=PW[:], in_=SC[:], axis=AX.X, op=OP.max)
                v.tensor_copy(out=PR[:], in_=CST[:, 0:32])
                v.tensor_scalar(out=P32[:, 0:1], in0=PW[:], scalar1=-1.0,
                                scalar2=31.0, op0=OP.mult, op1=OP.add)
                v.tensor_copy(out=CAND[:], in_=TEND[:])

                # scatter pointer at k == last_pos: bp' = bp + atlast*(ptr - bp)
                v.transpose(out=T32[:], in_=P32[:])
                v.stream_shuffle(out=PR[:], in_=T32[:], mask=[0] * 32)
                v.tensor_copy(out=CAND[:], in_=TEND[:])
                pr_b = PR[:, 0:4].unsqueeze(1).broadcast_to([128, K, 4])
                bp_v = SCR[:].rearrange("p (k b) -> p k b", b=4)
                xs_v = XS[:, 0:4 * K].rearrange("p (k b) -> p k b", b=4)
                xs2_v = XS2[:, 0:4 * K].rearrange("p (k b) -> p k b", b=4)
                alf_v = ALF[:, 0:4 * K].rearrange("p (k b) -> p k b", b=4)
                v.tensor_tensor(out=xs_v, in0=pr_b, in1=bp_v, op=OP.subtract)
                v.tensor_tensor(out=xs2_v, in0=xs_v, in1=alf_v, op=OP.mult)
                v.tensor_tensor(out=xs_v, in0=bp_v, in1=xs2_v, op=OP.add)

                # ---- backward pass ----
                # Bulk-transposed bp rows: one 4-block stream-transpose over
                # XS[:, 4k0 : 4k0+128] fills ring slots for ks {k0, k0+8,
                # k0+16, k0+24} at once (slot(k) = k % 32, 32 cols each).
                # The stt pointer-chase then runs as an ADJACENT chain
                # (151ns/step vs 613ns with per-step transposes); the 8
                # transposes of batch F-32 are emitted right after stt(F+m),
                # when all four ks sharing their slots are consumed.
                v.tensor_copy(out=DEC[:, S - 1:S], in_=P32[:, 0:1])
                tb32_r = TB32[:].rearrange("p (g m c) -> p g m c", g=4, m=8, c=32)

                def bigtr(k0):
                    v.transpose(out=tb32_r[:, :, k0 % 32, :],
                                in_=XS[:, 4 * k0:4 * k0 + 128].rearrange(
                                    "p (g c) -> p g c", c=32))

                for m in range(8):
                    bigtr(480 + m)
                # two independent partition-half chains (examples 0-7 vs
                # 8-15) interleaved: each chain's accum-write -> scalar-read
                # distance is 2 ops (fully adjacent stts race: the scalar
                # read is NOT interlocked against the accumulate retire)
                for k in range(S - 2, -1, -1):
                    sl = TB32[:, 32 * (k % 32):32 * (k % 32) + 32]
                    for h in (slice(0, 64), slice(64, 128)):
                        v.scalar_tensor_tensor(out=EQC[h, :], in0=CST[h, 0:32],
                                               scalar=DEC[h, k + 1:k + 2],
                                               in1=sl[h, :],
                                               op0=OP.is_equal, op1=OP.mult,
                                               accum_out=DEC[h, k:k + 1])
                    F = (k // 32) * 32
                    if k % 32 < 8 and F >= 32:
                        bigtr(F - 32 + (k % 32))

                v.tensor_copy(out=CAND[:], in_=TEND[:])
                v.tensor_copy(out=DECI[:], in_=DEC[:])

            @block.vector
            def _(v):
                v.wait_ge(dma_sem, 80)
                # reps as a HARDWARE loop: program size independent of reps
                # (body is idempotent -- all state re-initialized per pass)
                with v.register("rs") as rs:
                    v.reg_mov(rs, 0)
                    with v.Fori(0, reps):
                        emit_body(v, rs)
                v.drain().then_inc(done_sem, 1)

    return nc


def pack_inputs(feats, transitions, mask, S):
    """Host-side layout packing (pure data movement, no arithmetic beyond
    dtype conversion of the 0/1 mask)."""
    trans = np.ascontiguousarray(np.asarray(transitions, np.float32))
    ttrep = np.ascontiguousarray(np.tile(trans.T, (4, 1)))  # [128, 32]
    iota = np.arange(32, dtype=np.float32)
    cst = np.ascontiguousarray(
        np.tile(np.concatenate([iota, 31.0 - iota])[None, :], (128, 1)))
    # materialized descending iota over the (br, i) free layout: plain 2-dim
    # operand for the backpointer-encode multiply
    ct2 = np.ascontiguousarray(
        np.tile(np.tile(31.0 - iota, 4)[None, :], (128, 1)).astype(np.float32))
    in_maps = []
    bc = 16
    for c in range(NCORES):
        f = np.asarray(feats[bc * c:bc * c + bc], np.float32)  # [16, S, 32]
        ft = np.ascontiguousarray(
            f.reshape(4, 4, S, T).transpose(0, 3, 2, 1).reshape(128, 4 * S))
        m = np.asarray(mask[bc * c:bc * c + bc]).astype(np.float32)  # [16, S]
        mk = np.broadcast_to(
            m.reshape(4, 1, 4, S).transpose(0, 1, 3, 2), (4, 32, S, 4))
        mk = mk.reshape(128, 4 * S)
        mkp = np.zeros((128, 4 * S + 4), np.float32)
        mkp[:, :4 * S] = mk
        in_maps.append({"ft": ft, "mkf": mkp, "tt": ttrep, "cst": cst, "ct2": ct2})
    return in_maps


def unpack_outputs(results, S):
    out = np.empty((128, S), np.int32)
    bc = 16
    for c in range(NCORES):
        d = np.asarray(results[c]["dec"]).reshape(4, 32, S)
        out[bc * c:bc * c + bc] = d[:, 0:4, :].reshape(16, S)
    return out


_NC_CACHE = {}


def kernel(feats, transitions, mask):
    B, S, Tin = feats.shape
    assert (B, Tin) == (128, 32)
    if S not in _NC_CACHE:
        _NC_CACHE[S] = build_nc(S)
    nc = _NC_CACHE[S]
    in_maps = pack_inputs(feats, transitions, mask, S)
    res = run_bass_kernel_spmd(nc, in_maps, list(range(NCORES)))
    return unpack_outputs(res.results, S)


# revision 22
# speedup vs baseline: 1.1669x; 1.1669x over previous
"""Viterbi CRF decode on Trainium2 (Bass), 8-core data-parallel.

Problem: B=128, S=512, T=32 (30 labels + START=30, END=31).
  forward max-plus scan over S steps, backpointers, masked lengths,
  backward pointer-following pass. Output [B, S] int32 tag path.

Sharding: pure data parallel, 16 examples per core.

Per-core layout (SBUF partitions p = 32*q + j, quadrant q in [0,4) holds
examples b = 4q+br, br in [0,4); j in [0,32) is the tag index):
  - state P4[p, (br,i)] = part[b, i] (part vector replicated across the 32
    j-partitions of each quadrant)
  - forward step (5 DVE ops, fused dual reduce):
      RB[t%2] = [V_t (128) | W_{t-1} (128)] where V = scores + P4 and
      W = (V == part bcast) * (31 - i).
      RED2_t: one 8-segment max-reduce over RB[t%2] emits part_t AND the
      encoded backpointer row bpw_{t-1} into the interleaved history
      PHBW[p, 8t:8t+8] = [part_t (4), bpw_{t-1} (4)].
      eq_t, transpose_t (rebuild P4), mul_t, add_{t+1} fill the rest; the
      op order keeps >=1 op between every stream-shuffle write/read and
      the ALU ops that touch the same tensor (no HW write->read interlock).
  - pointer phase: arithmetic select of part at last valid position
    (monotone mask -> at-last indicator), argmax into END tag.
  - backward: per step, block-transpose of the bp row + one fused
    scalar_tensor_tensor (one-hot select, sum-accumulate) = the gather.

All compute on the vector engine (exact fp32, same association order as
the jax reference: (feats + trans) + part), DMA on sync engine.

The reps parameter is a HARDWARE loop bound (sequencer branch + register
counter): program size is independent of reps, so a reps=K vs reps=1
wall-clock difference isolates true device execution time of (K-1) body
passes instead of host-side per-instruction compile/serialize costs.
"""

import numpy as np
from contextlib import ExitStack

import concourse.bass as bass
import concourse.mybir as mybir
from concourse.bass_utils import run_bass_kernel_spmd

F32 = mybir.dt.float32
I32 = mybir.dt.int32
AX = mybir.AxisListType
OP = mybir.AluOpType

T = 32
START = 30
END = 31
NCORES = 8


def build_nc(S, reps=1, sim_compat=False):
    # Single compute engine (DVE) in program order: same-engine RAW/WAW is
    # serialized by the hardware (per-op pipe drain); the conservative race
    # detector does not model engine ordering, so it is disabled.
    nc = bass.Bass(detect_race_conditions=False)
    ft_d = nc.declare_dram_parameter("ft", [128, 4 * S], F32, isOutput=False)
    mkf_d = nc.declare_dram_parameter("mkf", [128, 4 * S + 4], F32, isOutput=False)
    tt_d = nc.declare_dram_parameter("tt", [128, 32], F32, isOutput=False)
    cst_d = nc.declare_dram_parameter("cst", [128, 64], F32, isOutput=False)
    ct2_d = nc.declare_dram_parameter("ct2", [128, 128], F32, isOutput=False)
    dec_d = nc.declare_dram_parameter("dec", [128, S], I32, isOutput=True)

    K = S - 1  # bp rows k in [0, K)

    with ExitStack() as ctx:
        def sb(name, shape, dt=F32):
            return ctx.enter_context(nc.sbuf_tensor(name, shape, dt))

        FT = sb("FT", [128, 4 * S])
        MKF = sb("MKF", [128, 4 * S + 4])
        TT = sb("TT", [128, 32])
        # interleaved history: [part_t (4) | bpw_{t-1} (4)] at cols 8t
        PHBW = sb("PHBW", [128, 8 * S + 16])
        RB0 = sb("RB0", [128, 256])  # [V | W] ping
        RB1 = sb("RB1", [128, 256])  # [V | W] pong
        EB = sb("EB", [128, 128])
        XS = sb("XS", [128, 4 * S + 32])
        XS2 = sb("XS2", [128, 4 * S + 32])
        SCR = sb("SCR", [128, 4 * K])
        ALF = sb("ALF", [128, 4 * S])
        ALB = sb("ALB", [128, 4 * S])
        SCH = sb("SCH", [128, 64 * 128])
        SCHB = sb("SCHB", [128, 64 * 128])
        CT2 = sb("CT2", [128, 128])
        P4 = sb("P4", [128, 128])
        T32 = sb("T32", [128, 32])
        DEC = sb("DEC", [128, S])
        DECI = sb("DECI", [128, S], I32)
        CST = sb("CST", [128, 64])
        TB32 = sb("TB32", [128, 1024])  # 32-slot ring of transposed bp rows
        TEND = sb("TEND", [128, 32])
        LPP = sb("LPP", [128, 32])
        TLP = sb("TLP", [128, 32])
        CAND = sb("CAND", [128, 32])
        MX = sb("MX", [128, 1])
        EQC = sb("EQC", [128, 32])
        PW = sb("PW", [128, 1])
        P32 = sb("P32", [128, 32])
        PR = sb("PR", [128, 32])
        SC = sb("SC", [128, 32])

        RB = [RB0, RB1]

        with (
            nc.semaphore() as dma_sem,
            nc.semaphore() as done_sem,
            nc.semaphore() as sch_sem,
            nc.semaphore() as cons_sem,
            nc.Block() as block,
        ):
            @block.sync
            def _(sync):
                sync.dma_start(out=FT[:], in_=ft_d[:]).then_inc(dma_sem, 16)
                sync.dma_start(out=MKF[:], in_=mkf_d[:]).then_inc(dma_sem, 16)
                sync.dma_start(out=TT[:], in_=tt_d[:]).then_inc(dma_sem, 16)
                sync.dma_start(out=CST[:], in_=cst_d[:]).then_inc(dma_sem, 16)
                sync.dma_start(out=CT2[:], in_=ct2_d[:]).then_inc(dma_sem, 16)
                sync.wait_ge(done_sem, 1)
                sync.dma_start(out=dec_d[:], in_=DECI[:]).then_inc(dma_sem, 16)

            # score-chunk producer: chunks 1..7 built on the (otherwise idle)
            # GPSIMD engine into a double buffer, paced by DVE consumption.
            # Register-held thresholds keep the monotonic semaphores correct
            # across reps-loop passes.
            tt_c = TT[:].unsqueeze(1).unsqueeze(1).broadcast_to([128, 64, 4, 32])

            def sch_src(c):
                return FT[:, 256 * c:256 * (c + 1)].rearrange(
                    "p (u b) -> p u b", b=4).unsqueeze(3).broadcast_to([128, 64, 4, 32])

            @block.gpsimd
            def _(g):
                g.wait_ge(dma_sem, 80)
                with g.register("rp") as rp:
                    g.reg_mov(rp, 0)
                    with g.Fori(0, reps):
                        # chunk c reuses the buffer of chunk c-2 (or, for
                        # c=1, of chunk 7 from the previous pass); DVE incs
                        # cons_sem once per consumed chunk in {0..5, 7}, so
                        # a wait at rp, rp+=1 per chunk lines up exactly.
                        for c in range(1, 8):
                            g.wait_ge(cons_sem, rp)
                            g.reg_add(rp, rp, 1)
                            buf = SCH if c % 2 == 0 else SCHB
                            g.tensor_tensor(
                                out=buf[:].rearrange("p (u b i) -> p u b i", b=4, i=32),
                                in0=sch_src(c), in1=tt_c, op=OP.add,
                            ).then_inc(sch_sem, 1)

            def emit_body(v, rs):
                # ---- constants / scratch init ----
                v.stream_shuffle(out=TEND[:], in_=TT[:], mask=[END] * 32)
                v.memset(XS[:, 4 * K:], 0.0)
                v.memset(RB0[:, 128:256], 0.0)
                v.memset(RB1[:, 128:256], 0.0)
                v.memset(P32[:], 0.0)
                v.memset(LPP[:], 0.0)

                # init t=0: part0[b, j] = feats[b,0,j] + trans[START, j]
                v.tensor_scalar_add(out=PHBW[:, 0:4], in0=FT[:, 0:4],
                                    scalar1=TT[:, START:START + 1])
                # independent fillers keep an op of distance between
                # end-of-stream writes and start-of-stream shuffle reads
                v.tensor_sub(out=ALF[:], in0=MKF[:, 0:4 * S], in1=MKF[:, 4:4 * S + 4])
                # chunk 0 of the scores is built on DVE (needed immediately;
                # the GPSIMD producer covers chunks 1..7)
                v.tensor_tensor(out=SCH[:].rearrange("p (u b i) -> p u b i", b=4, i=32),
                                in0=sch_src(0), in1=tt_c, op=OP.add)
                p4_blk = P4[:].rearrange("p (b i) -> p b i", b=4)

                def p4_build(t0):
                    # replicate part_t0[b,:] to every partition of its quadrant:
                    # one 4-block stream-transpose with 0-stride input columns
                    if sim_compat:
                        for br in range(4):
                            v.transpose(out=P4[:, 32 * br:32 * br + 32],
                                        in_=PHBW[:, 8 * t0 + br:8 * t0 + br + 1].broadcast_to([128, 32]))
                    else:
                        v.transpose(out=p4_blk,
                                    in_=PHBW[:, 8 * t0:8 * t0 + 4].unsqueeze(2).broadcast_to([128, 4, 32]))

                p4_build(0)
                v.tensor_scalar(out=ALB[:], in0=ALF[:], scalar1=1.0,
                                scalar2=1e30, op0=OP.subtract, op1=OP.mult)
                # pre-loop: V_1 = scores_1 + P4_0
                v.tensor_tensor(out=RB1[:, 0:128], in0=SCH[:, 128:256],
                                in1=P4[:], op=OP.add)

                eb_v = EB[:].rearrange("p (b i) -> p b i", b=4)

                # ---- forward scan: 5 ops per step ----
                for t in range(1, S - 1):
                    rb_c = RB[t % 2]
                    rb_n = RB[(t + 1) % 2]
                    v.tensor_reduce(out=PHBW[:, 8 * t:8 * t + 8],
                                    in_=rb_c[:].rearrange("p (s i) -> p s i", s=8),
                                    axis=AX.X, op=OP.max)
                    v.tensor_tensor(out=eb_v,
                                    in0=rb_c[:, 0:128].rearrange("p (b i) -> p b i", b=4),
                                    in1=PHBW[:, 8 * t:8 * t + 4].unsqueeze(2).broadcast_to([128, 4, 32]),
                                    op=OP.is_equal)
                    p4_build(t)
                    v.tensor_tensor(out=rb_n[:, 128:256],
                                    in0=EB[:], in1=CT2[:], op=OP.mult)
                    u1 = (t + 1) % 64
                    c1 = (t + 1) // 64
                    if u1 == 0:
                        # new chunk: wait for the GPSIMD producer
                        v.reg_add(rs, rs, 1)
                        v.wait_ge(sch_sem, rs)
                    buf = SCH if c1 % 2 == 0 else SCHB
                    ins = v.tensor_tensor(out=rb_n[:, 0:128],
                                          in0=buf[:, 128 * u1:128 * u1 + 128],
                                          in1=P4[:], op=OP.add)
                    if u1 == 63 and c1 != 6:
                        # chunk c1 fully consumed; free its buffer (chunk 6's
                        # buffer is next written by DVE itself, no sem needed)
                        ins.then_inc(cons_sem, 1)

                # tail t = S-1: last partition + last bp row
                tl = S - 1
                v.tensor_reduce(out=PHBW[:, 8 * tl:8 * tl + 8],
                                in_=RB[tl % 2][:].rearrange("p (s i) -> p s i", s=8),
                                axis=AX.X, op=OP.max)
                v.tensor_tensor(out=eb_v,
                                in0=RB[tl % 2][:, 0:128].rearrange("p (b i) -> p b i", b=4),
                                in1=PHBW[:, 8 * tl:8 * tl + 4].unsqueeze(2).broadcast_to([128, 4, 32]),
                                op=OP.is_equal)
                v.tensor_tensor(out=RB[S % 2][:, 128:256],
                                in0=EB[:], in1=CT2[:], op=OP.mult)
                v.tensor_reduce(out=PHBW[:, 8 * S + 4:8 * S + 8],
                                in_=RB[S % 2][:, 128:256].rearrange("p (b i) -> p b i", b=4),
                                axis=AX.X, op=OP.max)

                # ---- last_partition by-i-partition: max over t of PH + ALB ----
                ph_tb = PHBW[:, 0:8 * S].rearrange("p (t c) -> p t c", c=8)[:, :, 0:4]
                alb_tb = ALB[:].rearrange("p (t b) -> p t b", b=4)
                xs_tb = XS[:, 0:4 * S].rearrange("p (t b) -> p t b", b=4)
                v.tensor_tensor(out=xs_tb, in0=ph_tb, in1=alb_tb, op=OP.add)
                v.tensor_reduce(out=LPP[:, 0:4],
                                in_=XS[:, 0:4 * S].rearrange("p (t b) -> p b t", b=4),
                                axis=AX.X, op=OP.max)

                # bp decode + mask (independent of LPP; also serves as filler)
                # bp row k = bpw_{k+1} at PHBW cols 8k+20 .. 8k+24
                bp_src = PHBW[:, 16:16 + 8 * K].rearrange("p (k c) -> p k c", c=8)[:, :, 4:8]
                xs2_kb = XS2[:, 0:4 * K].rearrange("p (k b) -> p k b", b=4)
                scr_kb = SCR[:].rearrange("p (k b) -> p k b", b=4)
                mkf_kb = MKF[:, 4:4 * K + 4].rearrange("p (k b) -> p k b", b=4)
                v.tensor_scalar(out=xs2_kb, in0=bp_src,
                                scalar1=-1.0, scalar2=31.0, op0=OP.mult, op1=OP.add)

                # pointer = argmax_i(LP[b,i] + trans[i,END]); one-time tail.
                # Independent ops (incl. the relocated bp-mask bulk op) serve
                # as spacers between each end-write -> start-read pair --
                # explicit drains cost ~2us each on HW.
                v.transpose(out=TLP[:], in_=LPP[:])
                v.tensor_tensor(out=scr_kb, in0=xs2_kb, in1=mkf_kb, op=OP.mult)
                v.tensor_tensor(out=CAND[:], in0=TLP[:], in1=TEND[:], op=OP.add)
                v.tensor_reduce(out=MX[:], in_=CAND[:], axis=AX.X, op=OP.max)
                v.tensor_copy(out=PR[:], in_=CST[:, 0:32])
                v.tensor_tensor(out=EQC[:], in0=CAND[:],
                                in1=MX[:].broadcast_to([128, 32]), op=OP.is_equal)
                v.tensor_tensor(out=SC[:], in0=EQC[:], in1=CST[:, 32:64], op=OP.mult)
                v.tensor_reduce(out=PW[:], in_=SC[:], axis=AX.X, op=OP.max)
                v.tensor_copy(out=PR[:], in_=CST[:, 0:32])
                v.tensor_scalar(out=P32[:, 0:1], in0=PW[:], scalar1=-1.0,
                                scalar2=31.0, op0=OP.mult, op1=OP.add)
                v.tensor_copy(out=CAND[:], in_=TEND[:])

                # scatter pointer at k == last_pos: bp' = bp + atlast*(ptr - bp)
                v.transpose(out=T32[:], in_=P32[:])
                v.stream_shuffle(out=PR[:], in_=T32[:], mask=[0] * 32)
                v.tensor_copy(out=CAND[:], in_=TEND[:])
                pr_b = PR[:, 0:4].unsqueeze(1).broadcast_to([128, K, 4])
                bp_v = SCR[:].rearrange("p (k b) -> p k b", b=4)
                xs_v = XS[:, 0:4 * K].rearrange("p (k b) -> p k b", b=4)
                xs2_v = XS2[:, 0:4 * K].rearrange("p (k b) -> p k b", b=4)
                alf_v = ALF[:, 0:4 * K].rearrange("p (k b) -> p k b", b=4)
                v.tensor_tensor(out=xs_v, in0=pr_b, in1=bp_v, op=OP.subtract)
                v.tensor_tensor(out=xs2_v, in0=xs_v, in1=alf_v, op=OP.mult)
                v.tensor_tensor(out=xs_v, in0=bp_v, in1=xs2_v, op=OP.add)

                # ---- backward pass ----
                # Bulk-transposed bp rows: one 4-block stream-transpose over
                # XS[:, 4k0 : 4k0+128] fills ring slots for ks {k0, k0+8,
                # k0+16, k0+24} at once (slot(k) = k % 32, 32 cols each).
                # The stt pointer-chase then runs as an ADJACENT chain
                # (151ns/step vs 613ns with per-step transposes); the 8
                # transposes of batch F-32 are emitted right after stt(F+m),
                # when all four ks sharing their slots are consumed.
                v.tensor_copy(out=DEC[:, S - 1:S], in_=P32[:, 0:1])
                tb32_r = TB32[:].rearrange("p (g m c) -> p g m c", g=4, m=8, c=32)

                def bigtr(k0):
                    v.transpose(out=tb32_r[:, :, k0 % 32, :],
                                in_=XS[:, 4 * k0:4 * k0 + 128].rearrange(
                                    "p (g c) -> p g c", c=32))

                for m in range(8):
                    bigtr(480 + m)
                # stt pointer-chase with one independent spacer op between
                # consecutive stts: fully adjacent stts race (the scalar
                # read is NOT interlocked against the accumulate retire).
                # 8 of every 32 spacer slots are the useful batch
                # transposes; the rest are cheap dummy copies.
                for k in range(S - 2, -1, -1):
                    v.scalar_tensor_tensor(out=EQC[:], in0=CST[:, 0:32],
                                           scalar=DEC[:, k + 1:k + 2],
                                           in1=TB32[:, 32 * (k % 32):32 * (k % 32) + 32],
                                           op0=OP.is_equal, op1=OP.mult,
                                           accum_out=DEC[:, k:k + 1])
                    F = (k // 32) * 32
                    if k % 32 < 8 and F >= 32:
                        bigtr(F - 32 + (k % 32))
                    else:
                        v.tensor_copy(out=CAND[:], in_=CST[:, 0:32])

                v.tensor_copy(out=CAND[:], in_=TEND[:])
                v.tensor_copy(out=DECI[:], in_=DEC[:])

            @block.vector
            def _(v):
                v.wait_ge(dma_sem, 80)
                # reps as a HARDWARE loop: program size independent of reps
                # (body is idempotent -- all state re-initialized per pass)
                with v.register("rs") as rs:
                    v.reg_mov(rs, 0)
                    with v.Fori(0, reps):
                        emit_body(v, rs)
                v.drain().then_inc(done_sem, 1)

    return nc


def pack_inputs(feats, transitions, mask, S):
    """Host-side layout packing (pure data movement, no arithmetic beyond
    dtype conversion of the 0/1 mask)."""
    trans = np.ascontiguousarray(np.asarray(transitions, np.float32))
    ttrep = np.ascontiguousarray(np.tile(trans.T, (4, 1)))  # [128, 32]
    iota = np.arange(32, dtype=np.float32)
    cst = np.ascontiguousarray(
        np.tile(np.concatenate([iota, 31.0 - iota])[None, :], (128, 1)))
    # materialized descending iota over the (br, i) free layout: plain 2-dim
    # operand for the backpointer-encode multiply
    ct2 = np.ascontiguousarray(
        np.tile(np.tile(31.0 - iota, 4)[None, :], (128, 1)).astype(np.float32))
    in_maps = []
    bc = 16
    for c in range(NCORES):
        f = np.asarray(feats[bc * c:bc * c + bc], np.float32)  # [16, S, 32]
        ft = np.ascontiguousarray(
            f.reshape(4, 4, S, T).transpose(0, 3, 2, 1).reshape(128, 4 * S))
        m = np.asarray(mask[bc * c:bc * c + bc]).astype(np.float32)  # [16, S]
        mk = np.broadcast_to(
            m.reshape(4, 1, 4, S).transpose(0, 1, 3, 2), (4, 32, S, 4))
        mk = mk.reshape(128, 4 * S)
        mkp = np.zeros((128, 4 * S + 4), np.float32)
        mkp[:, :4 * S] = mk
        in_maps.append({"ft": ft, "mkf": mkp, "tt": ttrep, "cst": cst, "ct2": ct2})
    return in_maps


def unpack_outputs(results, S):
    out = np.empty((128, S), np.int32)
    bc = 16
    for c in range(NCORES):
        d = np.asarray(results[c]["dec"]).reshape(4, 32, S)
        out[bc * c:bc * c + bc] = d[:, 0:4, :].reshape(16, S)
    return out


_NC_CACHE = {}


def kernel(feats, transitions, mask):
    B, S, Tin = feats.shape
    assert (B, Tin) == (128, 32)
    if S not in _NC_CACHE:
        _NC_CACHE[S] = build_nc(S)
    nc = _NC_CACHE[S]
    in_maps = pack_inputs(feats, transitions, mask, S)
    res = run_bass_kernel_spmd(nc, in_maps, list(range(NCORES)))
    return unpack_outputs(res.results, S)


# revision 23
# speedup vs baseline: 1.3913x; 1.1922x over previous
"""Viterbi CRF decode on Trainium2 (Bass), 8-core data-parallel.

Problem: B=128, S=512, T=32 (30 labels + START=30, END=31).
  forward max-plus scan over S steps, backpointers, masked lengths,
  backward pointer-following pass. Output [B, S] int32 tag path.

Sharding: pure data parallel, 16 examples per core.

Per-core layout (SBUF partitions p = 32*q + j, quadrant q in [0,4) holds
examples b = 4q+br, br in [0,4); j in [0,32) is the tag index):
  - state P4[p, (br,i)] = part[b, i] (part vector replicated across the 32
    j-partitions of each quadrant)
  - forward step (5 DVE ops, fused dual reduce):
      RB[t%2] = [V_t (128) | W_{t-1} (128)] where V = scores + P4 and
      W = (V == part bcast) * (31 - i).
      RED2_t: one 8-segment max-reduce over RB[t%2] emits part_t AND the
      encoded backpointer row bpw_{t-1} into the interleaved history
      PHBW[p, 8t:8t+8] = [part_t (4), bpw_{t-1} (4)].
      eq_t, transpose_t (rebuild P4), mul_t, add_{t+1} fill the rest; the
      op order keeps >=1 op between every stream-shuffle write/read and
      the ALU ops that touch the same tensor (no HW write->read interlock).
  - pointer phase: arithmetic select of part at last valid position
    (monotone mask -> at-last indicator), argmax into END tag.
  - backward: per step, block-transpose of the bp row + one fused
    scalar_tensor_tensor (one-hot select, sum-accumulate) = the gather.

All compute on the vector engine (exact fp32, same association order as
the jax reference: (feats + trans) + part), DMA on sync engine.

The reps parameter is a HARDWARE loop bound (sequencer branch + register
counter): program size is independent of reps, so a reps=K vs reps=1
wall-clock difference isolates true device execution time of (K-1) body
passes instead of host-side per-instruction compile/serialize costs.
"""

import numpy as np
from contextlib import ExitStack

import concourse.bass as bass
import concourse.mybir as mybir
from concourse.bass_utils import run_bass_kernel_spmd

F32 = mybir.dt.float32
I32 = mybir.dt.int32
AX = mybir.AxisListType
OP = mybir.AluOpType

T = 32
START = 30
END = 31
NCORES = 8


def build_nc(S, reps=1, sim_compat=False):
    # Single compute engine (DVE) in program order: same-engine RAW/WAW is
    # serialized by the hardware (per-op pipe drain); the conservative race
    # detector does not model engine ordering, so it is disabled.
    nc = bass.Bass(detect_race_conditions=False)
    ft_d = nc.declare_dram_parameter("ft", [128, 4 * S], F32, isOutput=False)
    mkf_d = nc.declare_dram_parameter("mkf", [128, 4 * S + 4], F32, isOutput=False)
    tt_d = nc.declare_dram_parameter("tt", [128, 32], F32, isOutput=False)
    cst_d = nc.declare_dram_parameter("cst", [128, 64], F32, isOutput=False)
    ct2_d = nc.declare_dram_parameter("ct2", [128, 128], F32, isOutput=False)
    dec_d = nc.declare_dram_parameter("dec", [128, S], I32, isOutput=True)

    K = S - 1  # bp rows k in [0, K)

    with ExitStack() as ctx:
        def sb(name, shape, dt=F32):
            return ctx.enter_context(nc.sbuf_tensor(name, shape, dt))

        FT = sb("FT", [128, 4 * S])
        MKF = sb("MKF", [128, 4 * S + 4])
        TT = sb("TT", [128, 32])
        # interleaved history: [part_t (4) | bpw_{t-1} (4)] at cols 8t
        PHBW = sb("PHBW", [128, 8 * S + 16])
        RB0 = sb("RB0", [128, 256])  # [V | W] ping
        RB1 = sb("RB1", [128, 256])  # [V | W] pong
        EB = sb("EB", [128, 128])
        XS = sb("XS", [128, 4 * S + 32])
        XS2 = sb("XS2", [128, 4 * S + 32])
        SCR = sb("SCR", [128, 4 * K])
        ALF = sb("ALF", [128, 4 * S])
        ALB = sb("ALB", [128, 4 * S])
        SCH = sb("SCH", [128, 64 * 128])
        SCHB = sb("SCHB", [128, 64 * 128])
        CT2 = sb("CT2", [128, 128])
        P4 = sb("P4", [128, 128])
        T32 = sb("T32", [128, 32])
        DEC = sb("DEC", [128, S])
        DECI = sb("DECI", [128, S], I32)
        CST = sb("CST", [128, 64])
        TB32 = sb("TB32", [128, 1024])  # 32-slot ring of transposed bp rows
        TEND = sb("TEND", [128, 32])
        LPP = sb("LPP", [128, 32])
        TLP = sb("TLP", [128, 32])
        CAND = sb("CAND", [128, 32])
        MX = sb("MX", [128, 1])
        EQC = sb("EQC", [128, 32])
        PW = sb("PW", [128, 1])
        P32 = sb("P32", [128, 32])
        PR = sb("PR", [128, 32])
        SC = sb("SC", [128, 32])

        RB = [RB0, RB1]

        with (
            nc.semaphore() as dma_sem,
            nc.semaphore() as done_sem,
            nc.semaphore() as sch_sem,
            nc.semaphore() as cons_sem,
            nc.Block() as block,
        ):
            @block.sync
            def _(sync):
                sync.dma_start(out=FT[:], in_=ft_d[:]).then_inc(dma_sem, 16)
                sync.dma_start(out=MKF[:], in_=mkf_d[:]).then_inc(dma_sem, 16)
                sync.dma_start(out=TT[:], in_=tt_d[:]).then_inc(dma_sem, 16)
                sync.dma_start(out=CST[:], in_=cst_d[:]).then_inc(dma_sem, 16)
                sync.dma_start(out=CT2[:], in_=ct2_d[:]).then_inc(dma_sem, 16)
                sync.wait_ge(done_sem, 1)
                sync.dma_start(out=dec_d[:], in_=DECI[:]).then_inc(dma_sem, 16)

            # score-chunk producer: chunks 1..7 built on the (otherwise idle)
            # GPSIMD engine into a double buffer, paced by DVE consumption.
            # Register-held thresholds keep the monotonic semaphores correct
            # across reps-loop passes.
            tt_c = TT[:].unsqueeze(1).unsqueeze(1).broadcast_to([128, 64, 4, 32])

            def sch_src(c):
                return FT[:, 256 * c:256 * (c + 1)].rearrange(
                    "p (u b) -> p u b", b=4).unsqueeze(3).broadcast_to([128, 64, 4, 32])

            @block.gpsimd
            def _(g):
                g.wait_ge(dma_sem, 80)
                with g.register("rp") as rp:
                    g.reg_mov(rp, 0)
                    with g.Fori(0, reps):
                        # chunk c reuses the buffer of chunk c-2 (or, for
                        # c=1, of chunk 7 from the previous pass); DVE incs
                        # cons_sem once per consumed chunk in {0..5, 7}, so
                        # a wait at rp, rp+=1 per chunk lines up exactly.
                        for c in range(1, 8):
                            g.wait_ge(cons_sem, rp)
                            g.reg_add(rp, rp, 1)
                            buf = SCH if c % 2 == 0 else SCHB
                            g.tensor_tensor(
                                out=buf[:].rearrange("p (u b i) -> p u b i", b=4, i=32),
                                in0=sch_src(c), in1=tt_c, op=OP.add,
                            ).then_inc(sch_sem, 1)

            def emit_body(v, rs):
                # ---- constants / scratch init ----
                v.stream_shuffle(out=TEND[:], in_=TT[:], mask=[END] * 32)
                v.memset(XS[:, 4 * K:], 0.0)
                v.memset(RB0[:, 128:256], 0.0)
                v.memset(RB1[:, 128:256], 0.0)
                v.memset(P32[:], 0.0)
                v.memset(LPP[:], 0.0)

                # init t=0: part0[b, j] = feats[b,0,j] + trans[START, j]
                v.tensor_scalar_add(out=PHBW[:, 0:4], in0=FT[:, 0:4],
                                    scalar1=TT[:, START:START + 1])
                # independent fillers keep an op of distance between
                # end-of-stream writes and start-of-stream shuffle reads
                v.tensor_sub(out=ALF[:], in0=MKF[:, 0:4 * S], in1=MKF[:, 4:4 * S + 4])
                # chunk 0 of the scores is built on DVE (needed immediately;
                # the GPSIMD producer covers chunks 1..7)
                v.tensor_tensor(out=SCH[:].rearrange("p (u b i) -> p u b i", b=4, i=32),
                                in0=sch_src(0), in1=tt_c, op=OP.add)
                p4_blk = P4[:].rearrange("p (b i) -> p b i", b=4)

                def p4_build(t0):
                    # replicate part_t0[b,:] to every partition of its quadrant:
                    # one 4-block stream-transpose with 0-stride input columns
                    if sim_compat:
                        for br in range(4):
                            v.transpose(out=P4[:, 32 * br:32 * br + 32],
                                        in_=PHBW[:, 8 * t0 + br:8 * t0 + br + 1].broadcast_to([128, 32]))
                    else:
                        v.transpose(out=p4_blk,
                                    in_=PHBW[:, 8 * t0:8 * t0 + 4].unsqueeze(2).broadcast_to([128, 4, 32]))

                p4_build(0)
                v.tensor_scalar(out=ALB[:], in0=ALF[:], scalar1=1.0,
                                scalar2=1e30, op0=OP.subtract, op1=OP.mult)
                # pre-loop: V_1 = scores_1 + P4_0
                v.tensor_tensor(out=RB1[:, 0:128], in0=SCH[:, 128:256],
                                in1=P4[:], op=OP.add)

                eb_v = EB[:].rearrange("p (b i) -> p b i", b=4)

                # ---- forward scan: 5 ops per step ----
                for t in range(1, S - 1):
                    rb_c = RB[t % 2]
                    rb_n = RB[(t + 1) % 2]
                    v.tensor_reduce(out=PHBW[:, 8 * t:8 * t + 8],
                                    in_=rb_c[:].rearrange("p (s i) -> p s i", s=8),
                                    axis=AX.X, op=OP.max)
                    v.tensor_tensor(out=eb_v,
                                    in0=rb_c[:, 0:128].rearrange("p (b i) -> p b i", b=4),
                                    in1=PHBW[:, 8 * t:8 * t + 4].unsqueeze(2).broadcast_to([128, 4, 32]),
                                    op=OP.is_equal)
                    p4_build(t)
                    v.tensor_tensor(out=rb_n[:, 128:256],
                                    in0=EB[:], in1=CT2[:], op=OP.mult)
                    u1 = (t + 1) % 64
                    c1 = (t + 1) // 64
                    if u1 == 0:
                        # new chunk: wait for the GPSIMD producer
                        v.reg_add(rs, rs, 1)
                        v.wait_ge(sch_sem, rs)
                    buf = SCH if c1 % 2 == 0 else SCHB
                    ins = v.tensor_tensor(out=rb_n[:, 0:128],
                                          in0=buf[:, 128 * u1:128 * u1 + 128],
                                          in1=P4[:], op=OP.add)
                    if u1 == 63 and c1 != 6:
                        # chunk c1 fully consumed; free its buffer (chunk 6's
                        # buffer is next written by DVE itself, no sem needed)
                        ins.then_inc(cons_sem, 1)

                # tail t = S-1: last partition + last bp row
                tl = S - 1
                v.tensor_reduce(out=PHBW[:, 8 * tl:8 * tl + 8],
                                in_=RB[tl % 2][:].rearrange("p (s i) -> p s i", s=8),
                                axis=AX.X, op=OP.max)
                v.tensor_tensor(out=eb_v,
                                in0=RB[tl % 2][:, 0:128].rearrange("p (b i) -> p b i", b=4),
                                in1=PHBW[:, 8 * tl:8 * tl + 4].unsqueeze(2).broadcast_to([128, 4, 32]),
                                op=OP.is_equal)
                v.tensor_tensor(out=RB[S % 2][:, 128:256],
                                in0=EB[:], in1=CT2[:], op=OP.mult)
                v.tensor_reduce(out=PHBW[:, 8 * S + 4:8 * S + 8],
                                in_=RB[S % 2][:, 128:256].rearrange("p (b i) -> p b i", b=4),
                                axis=AX.X, op=OP.max)

                # ---- last_partition by-i-partition: max over t of PH + ALB ----
                ph_tb = PHBW[:, 0:8 * S].rearrange("p (t c) -> p t c", c=8)[:, :, 0:4]
                alb_tb = ALB[:].rearrange("p (t b) -> p t b", b=4)
                xs_tb = XS[:, 0:4 * S].rearrange("p (t b) -> p t b", b=4)
                v.tensor_tensor(out=xs_tb, in0=ph_tb, in1=alb_tb, op=OP.add)
                v.tensor_reduce(out=LPP[:, 0:4],
                                in_=XS[:, 0:4 * S].rearrange("p (t b) -> p b t", b=4),
                                axis=AX.X, op=OP.max)

                # bp decode + mask (independent of LPP; also serves as filler)
                # bp row k = bpw_{k+1} at PHBW cols 8k+20 .. 8k+24
                bp_src = PHBW[:, 16:16 + 8 * K].rearrange("p (k c) -> p k c", c=8)[:, :, 4:8]
                xs2_kb = XS2[:, 0:4 * K].rearrange("p (k b) -> p k b", b=4)
                scr_kb = SCR[:].rearrange("p (k b) -> p k b", b=4)
                mkf_kb = MKF[:, 4:4 * K + 4].rearrange("p (k b) -> p k b", b=4)
                v.tensor_scalar(out=xs2_kb, in0=bp_src,
                                scalar1=-1.0, scalar2=31.0, op0=OP.mult, op1=OP.add)

                # pointer = argmax_i(LP[b,i] + trans[i,END]); one-time tail.
                # Independent ops (incl. the relocated bp-mask bulk op) serve
                # as spacers between each end-write -> start-read pair --
                # explicit drains cost ~2us each on HW.
                v.transpose(out=TLP[:], in_=LPP[:])
                v.tensor_tensor(out=scr_kb, in0=xs2_kb, in1=mkf_kb, op=OP.mult)
                v.tensor_tensor(out=CAND[:], in0=TLP[:], in1=TEND[:], op=OP.add)
                v.tensor_reduce(out=MX[:], in_=CAND[:], axis=AX.X, op=OP.max)
                v.tensor_copy(out=PR[:], in_=CST[:, 0:32])
                v.tensor_tensor(out=EQC[:], in0=CAND[:],
                                in1=MX[:].broadcast_to([128, 32]), op=OP.is_equal)
                v.tensor_tensor(out=SC[:], in0=EQC[:], in1=CST[:, 32:64], op=OP.mult)
                v.tensor_reduce(out=PW[:], in_=SC[:], axis=AX.X, op=OP.max)
                v.tensor_copy(out=PR[:], in_=CST[:, 0:32])
                v.tensor_scalar(out=P32[:, 0:1], in0=PW[:], scalar1=-1.0,
                                scalar2=31.0, op0=OP.mult, op1=OP.add)
                v.tensor_copy(out=CAND[:], in_=TEND[:])

                # scatter pointer at k == last_pos: bp' = bp + atlast*(ptr - bp)
                v.transpose(out=T32[:], in_=P32[:])
                v.stream_shuffle(out=PR[:], in_=T32[:], mask=[0] * 32)
                v.tensor_copy(out=CAND[:], in_=TEND[:])
                pr_b = PR[:, 0:4].unsqueeze(1).broadcast_to([128, K, 4])
                bp_v = SCR[:].rearrange("p (k b) -> p k b", b=4)
                xs_v = XS[:, 0:4 * K].rearrange("p (k b) -> p k b", b=4)
                xs2_v = XS2[:, 0:4 * K].rearrange("p (k b) -> p k b", b=4)
                alf_v = ALF[:, 0:4 * K].rearrange("p (k b) -> p k b", b=4)
                v.tensor_tensor(out=xs_v, in0=pr_b, in1=bp_v, op=OP.subtract)
                v.tensor_tensor(out=xs2_v, in0=xs_v, in1=alf_v, op=OP.mult)
                v.tensor_tensor(out=xs_v, in0=bp_v, in1=xs2_v, op=OP.add)

                # ---- backward pass ----
                # Bulk-transposed bp rows: one 4-block stream-transpose over
                # XS[:, 4k0 : 4k0+128] fills ring slots for ks {k0, k0+8,
                # k0+16, k0+24} at once (slot(k) = k % 32, 32 cols each).
                # The stt pointer-chase then runs as an ADJACENT chain
                # (151ns/step vs 613ns with per-step transposes); the 8
                # transposes of batch F-32 are emitted right after stt(F+m),
                # when all four ks sharing their slots are consumed.
                v.tensor_copy(out=DEC[:, S - 1:S], in_=P32[:, 0:1])
                tb32_r = TB32[:].rearrange("p (g m c) -> p g m c", g=4, m=8, c=32)

                def bigtr(k0):
                    if sim_compat:
                        # CoreSim can't execute blocked multi-tile transposes;
                        # emit the 4 32x32 block transposes separately
                        for g in range(4):
                            v.transpose(
                                out=TB32[:, 32 * (k0 % 32 + 8 * g):32 * (k0 % 32 + 8 * g) + 32],
                                in_=XS[:, 4 * k0 + 32 * g:4 * k0 + 32 * g + 32])
                    else:
                        v.transpose(out=tb32_r[:, :, k0 % 32, :],
                                    in_=XS[:, 4 * k0:4 * k0 + 128].rearrange(
                                        "p (g c) -> p g c", c=32))

                for m in range(8):
                    bigtr(480 + m)
                # stt pointer-chase with one independent spacer op between
                # consecutive stts: fully adjacent stts race (the scalar
                # read is NOT interlocked against the accumulate retire).
                # 8 of every 32 spacer slots are the useful batch
                # transposes; the rest are cheap dummy copies.
                for k in range(S - 2, -1, -1):
                    v.scalar_tensor_tensor(out=EQC[:], in0=CST[:, 0:32],
                                           scalar=DEC[:, k + 1:k + 2],
                                           in1=TB32[:, 32 * (k % 32):32 * (k % 32) + 32],
                                           op0=OP.is_equal, op1=OP.mult,
                                           accum_out=DEC[:, k:k + 1])
                    F = (k // 32) * 32
                    if k % 32 < 8 and F >= 32:
                        bigtr(F - 32 + (k % 32))
                    else:
                        v.tensor_copy(out=CAND[:], in_=CST[:, 0:32])

                v.tensor_copy(out=CAND[:], in_=TEND[:])
                v.tensor_copy(out=DECI[:], in_=DEC[:])

            @block.vector
            def _(v):
                v.wait_ge(dma_sem, 80)
                # reps as a HARDWARE loop: program size independent of reps
                # (body is idempotent -- all state re-initialized per pass)
                with v.register("rs") as rs:
                    v.reg_mov(rs, 0)
                    with v.Fori(0, reps):
                        emit_body(v, rs)
                v.drain().then_inc(done_sem, 1)

    return nc


def pack_inputs(feats, transitions, mask, S):
    """Host-side layout packing (pure data movement, no arithmetic beyond
    dtype conversion of the 0/1 mask)."""
    trans = np.ascontiguousarray(np.asarray(transitions, np.float32))
    ttrep = np.ascontiguousarray(np.tile(trans.T, (4, 1)))  # [128, 32]
    iota = np.arange(32, dtype=np.float32)
    cst = np.ascontiguousarray(
        np.tile(np.concatenate([iota, 31.0 - iota])[None, :], (128, 1)))
    # materialized descending iota over the (br, i) free layout: plain 2-dim
    # operand for the backpointer-encode multiply
    ct2 = np.ascontiguousarray(
        np.tile(np.tile(31.0 - iota, 4)[None, :], (128, 1)).astype(np.float32))
    in_maps = []
    bc = 16
    for c in range(NCORES):
        f = np.asarray(feats[bc * c:bc * c + bc], np.float32)  # [16, S, 32]
        ft = np.ascontiguousarray(
            f.reshape(4, 4, S, T).transpose(0, 3, 2, 1).reshape(128, 4 * S))
        m = np.asarray(mask[bc * c:bc * c + bc]).astype(np.float32)  # [16, S]
        mk = np.broadcast_to(
            m.reshape(4, 1, 4, S).transpose(0, 1, 3, 2), (4, 32, S, 4))
        mk = mk.reshape(128, 4 * S)
        mkp = np.zeros((128, 4 * S + 4), np.float32)
        mkp[:, :4 * S] = mk
        in_maps.append({"ft": ft, "mkf": mkp, "tt": ttrep, "cst": cst, "ct2": ct2})
    return in_maps


def unpack_outputs(results, S):
    out = np.empty((128, S), np.int32)
    bc = 16
    for c in range(NCORES):
        d = np.asarray(results[c]["dec"]).reshape(4, 32, S)
        out[bc * c:bc * c + bc] = d[:, 0:4, :].reshape(16, S)
    return out


_NC_CACHE = {}


def kernel(feats, transitions, mask):
    B, S, Tin = feats.shape
    assert (B, Tin) == (128, 32)
    if S not in _NC_CACHE:
        _NC_CACHE[S] = build_nc(S)
    nc = _NC_CACHE[S]
    in_maps = pack_inputs(feats, transitions, mask, S)
    res = run_bass_kernel_spmd(nc, in_maps, list(range(NCORES)))
    return unpack_outputs(res.results, S)


# revision 26
# speedup vs baseline: 1.4961x; 1.0753x over previous
"""Viterbi CRF decode on Trainium2 (Bass), 8-core data-parallel.

Problem: B=128, S=512, T=32 (30 labels + START=30, END=31).
  forward max-plus scan over S steps, backpointers, masked lengths,
  backward pointer-following pass. Output [B, S] int32 tag path.

Sharding: pure data parallel, 16 examples per core.

Per-core layout (SBUF partitions p = 32*q + j, quadrant q in [0,4) holds
examples b = 4q+br, br in [0,4); j in [0,32) is the tag index):
  - state P4[p, (br,i)] = part[b, i] (part vector replicated across the 32
    j-partitions of each quadrant)
  - forward step (5 DVE ops, fused dual reduce):
      RB[t%2] = [V_t (128) | W_{t-1} (128)] where V = scores + P4 and
      W = (V == part bcast) * (31 - i).
      RED2_t: one 8-segment max-reduce over RB[t%2] emits part_t AND the
      encoded backpointer row bpw_{t-1} into the interleaved history
      PHBW[p, 8t:8t+8] = [part_t (4), bpw_{t-1} (4)].
      eq_t, transpose_t (rebuild P4), mul_t, add_{t+1} fill the rest; the
      op order keeps >=1 op between every stream-shuffle write/read and
      the ALU ops that touch the same tensor (no HW write->read interlock).
  - pointer phase: arithmetic select of part at last valid position
    (monotone mask -> at-last indicator), argmax into END tag.
  - backward: per step, block-transpose of the bp row + one fused
    scalar_tensor_tensor (one-hot select, sum-accumulate) = the gather.

All compute on the vector engine (exact fp32, same association order as
the jax reference: (feats + trans) + part), DMA on sync engine.

The reps parameter is a HARDWARE loop bound (sequencer branch + register
counter): program size is independent of reps, so a reps=K vs reps=1
wall-clock difference isolates true device execution time of (K-1) body
passes instead of host-side per-instruction compile/serialize costs.
"""

import numpy as np
from contextlib import ExitStack

import concourse.bass as bass
import concourse.mybir as mybir
from concourse.bass_utils import run_bass_kernel_spmd

F32 = mybir.dt.float32
I32 = mybir.dt.int32
AX = mybir.AxisListType
OP = mybir.AluOpType

T = 32
START = 30
END = 31
NCORES = 8


def build_nc(S, reps=1, sim_compat=False):
    # Single compute engine (DVE) in program order: same-engine RAW/WAW is
    # serialized by the hardware (per-op pipe drain); the conservative race
    # detector does not model engine ordering, so it is disabled.
    nc = bass.Bass(detect_race_conditions=False)
    ft_d = nc.declare_dram_parameter("ft", [128, 4 * S], F32, isOutput=False)
    mkf_d = nc.declare_dram_parameter("mkf", [128, 4 * S + 4], F32, isOutput=False)
    tt_d = nc.declare_dram_parameter("tt", [128, 32], F32, isOutput=False)
    cst_d = nc.declare_dram_parameter("cst", [128, 64], F32, isOutput=False)
    ct2_d = nc.declare_dram_parameter("ct2", [128, 128], F32, isOutput=False)
    dec_d = nc.declare_dram_parameter("dec", [128, S], I32, isOutput=True)

    K = S - 1  # bp rows k in [0, K)

    with ExitStack() as ctx:
        def sb(name, shape, dt=F32):
            return ctx.enter_context(nc.sbuf_tensor(name, shape, dt))

        FT = sb("FT", [128, 4 * S])
        MKF = sb("MKF", [128, 4 * S + 4])
        TT = sb("TT", [128, 32])
        # interleaved history: [part_t (4) | bpw_{t-1} (4)] at cols 8t
        PHBW = sb("PHBW", [128, 8 * S + 16])
        RB0 = sb("RB0", [128, 256])  # [V | W] ping
        RB1 = sb("RB1", [128, 256])  # [V | W] pong
        EB = sb("EB", [128, 128])
        XS = sb("XS", [128, 4 * S + 32])
        XS2 = sb("XS2", [128, 4 * S + 32])
        SCR = sb("SCR", [128, 4 * K])
        ALF = sb("ALF", [128, 4 * S])
        ALB = sb("ALB", [128, 4 * S])
        SCH = sb("SCH", [128, 64 * 128])
        CT2 = sb("CT2", [128, 128])
        P4 = sb("P4", [128, 128])
        T32 = sb("T32", [128, 32])
        DEC = sb("DEC", [128, S])
        DECI = sb("DECI", [128, S], I32)
        CST = sb("CST", [128, 64])
        TB32 = sb("TB32", [128, 1024])  # 32-slot ring of transposed bp rows
        TEND = sb("TEND", [128, 32])
        LPP = sb("LPP", [128, 32])
        TLP = sb("TLP", [128, 32])
        CAND = sb("CAND", [128, 32])
        MX = sb("MX", [128, 1])
        EQC = sb("EQC", [128, 32])
        PW = sb("PW", [128, 1])
        P32 = sb("P32", [128, 32])
        PR = sb("PR", [128, 32])
        SC = sb("SC", [128, 32])

        RB = [RB0, RB1]

        with (
            nc.semaphore() as dma_sem,
            nc.semaphore() as done_sem,
            nc.Block() as block,
        ):
            @block.sync
            def _(sync):
                sync.dma_start(out=FT[:], in_=ft_d[:]).then_inc(dma_sem, 16)
                sync.dma_start(out=MKF[:], in_=mkf_d[:]).then_inc(dma_sem, 16)
                sync.dma_start(out=TT[:], in_=tt_d[:]).then_inc(dma_sem, 16)
                sync.dma_start(out=CST[:], in_=cst_d[:]).then_inc(dma_sem, 16)
                sync.dma_start(out=CT2[:], in_=ct2_d[:]).then_inc(dma_sem, 16)
                sync.wait_ge(done_sem, 1)
                sync.dma_start(out=dec_d[:], in_=DECI[:]).then_inc(dma_sem, 16)

            # bulk score views: SCH[p, 128u + 32br + i] = feats[b,t0+u,j]
            # + trans[i,j], built one 64-step chunk at a time
            tt_c = TT[:].unsqueeze(1).unsqueeze(1).broadcast_to([128, 64, 4, 32])

            def sch_src(c):
                return FT[:, 256 * c:256 * (c + 1)].rearrange(
                    "p (u b) -> p u b", b=4).unsqueeze(3).broadcast_to([128, 64, 4, 32])

            def emit_body(v):
                # ---- constants / scratch init ----
                v.stream_shuffle(out=TEND[:], in_=TT[:], mask=[END] * 32)
                v.memset(XS[:, 4 * K:], 0.0)
                v.memset(RB0[:, 128:256], 0.0)
                v.memset(RB1[:, 128:256], 0.0)
                v.memset(P32[:], 0.0)
                v.memset(LPP[:], 0.0)

                # init t=0: part0[b, j] = feats[b,0,j] + trans[START, j]
                v.tensor_scalar_add(out=PHBW[:, 0:4], in0=FT[:, 0:4],
                                    scalar1=TT[:, START:START + 1])
                # independent fillers keep an op of distance between
                # end-of-stream writes and start-of-stream shuffle reads
                v.tensor_sub(out=ALF[:], in0=MKF[:, 0:4 * S], in1=MKF[:, 4:4 * S + 4])
                v.tensor_tensor(out=SCH[:].rearrange("p (u b i) -> p u b i", b=4, i=32),
                                in0=sch_src(0), in1=tt_c, op=OP.add)
                p4_blk = P4[:].rearrange("p (b i) -> p b i", b=4)

                def p4_build(t0):
                    # replicate part_t0[b,:] to every partition of its quadrant:
                    # one 4-block stream-transpose with 0-stride input columns
                    if sim_compat:
                        for br in range(4):
                            v.transpose(out=P4[:, 32 * br:32 * br + 32],
                                        in_=PHBW[:, 8 * t0 + br:8 * t0 + br + 1].broadcast_to([128, 32]))
                    else:
                        v.transpose(out=p4_blk,
                                    in_=PHBW[:, 8 * t0:8 * t0 + 4].unsqueeze(2).broadcast_to([128, 4, 32]))

                p4_build(0)
                v.tensor_scalar(out=ALB[:], in0=ALF[:], scalar1=1.0,
                                scalar2=1e30, op0=OP.subtract, op1=OP.mult)
                # pre-loop: V_1 = scores_1 + P4_0
                v.tensor_tensor(out=RB1[:, 0:128], in0=SCH[:, 128:256],
                                in1=P4[:], op=OP.add)

                eb_v = EB[:].rearrange("p (b i) -> p b i", b=4)

                # ---- forward scan: 5 ops per step ----
                for t in range(1, S - 1):
                    rb_c = RB[t % 2]
                    rb_n = RB[(t + 1) % 2]
                    v.tensor_reduce(out=PHBW[:, 8 * t:8 * t + 8],
                                    in_=rb_c[:].rearrange("p (s i) -> p s i", s=8),
                                    axis=AX.X, op=OP.max)
                    v.tensor_tensor(out=eb_v,
                                    in0=rb_c[:, 0:128].rearrange("p (b i) -> p b i", b=4),
                                    in1=PHBW[:, 8 * t:8 * t + 4].unsqueeze(2).broadcast_to([128, 4, 32]),
                                    op=OP.is_equal)
                    p4_build(t)
                    v.tensor_tensor(out=rb_n[:, 128:256],
                                    in0=EB[:], in1=CT2[:], op=OP.mult)
                    u1 = (t + 1) % 64
                    c1 = (t + 1) // 64
                    if u1 == 0:
                        # next 64-step score chunk, built inline on DVE (a
                        # concurrent GPSIMD producer measures ~90us SLOWER:
                        # its big streaming op contends for SBUF with every
                        # small DVE op of the scan)
                        v.tensor_tensor(
                            out=SCH[:].rearrange("p (u b i) -> p u b i", b=4, i=32),
                            in0=sch_src(c1), in1=tt_c, op=OP.add)
                    v.tensor_tensor(out=rb_n[:, 0:128],
                                    in0=SCH[:, 128 * u1:128 * u1 + 128],
                                    in1=P4[:], op=OP.add)

                # tail t = S-1: last partition + last bp row
                tl = S - 1
                v.tensor_reduce(out=PHBW[:, 8 * tl:8 * tl + 8],
                                in_=RB[tl % 2][:].rearrange("p (s i) -> p s i", s=8),
                                axis=AX.X, op=OP.max)
                v.tensor_tensor(out=eb_v,
                                in0=RB[tl % 2][:, 0:128].rearrange("p (b i) -> p b i", b=4),
                                in1=PHBW[:, 8 * tl:8 * tl + 4].unsqueeze(2).broadcast_to([128, 4, 32]),
                                op=OP.is_equal)
                v.tensor_tensor(out=RB[S % 2][:, 128:256],
                                in0=EB[:], in1=CT2[:], op=OP.mult)
                v.tensor_reduce(out=PHBW[:, 8 * S + 4:8 * S + 8],
                                in_=RB[S % 2][:, 128:256].rearrange("p (b i) -> p b i", b=4),
                                axis=AX.X, op=OP.max)

                # ---- last_partition by-i-partition: max over t of PH + ALB ----
                ph_tb = PHBW[:, 0:8 * S].rearrange("p (t c) -> p t c", c=8)[:, :, 0:4]
                alb_tb = ALB[:].rearrange("p (t b) -> p t b", b=4)
                xs_tb = XS[:, 0:4 * S].rearrange("p (t b) -> p t b", b=4)
                v.tensor_tensor(out=xs_tb, in0=ph_tb, in1=alb_tb, op=OP.add)
                v.tensor_reduce(out=LPP[:, 0:4],
                                in_=XS[:, 0:4 * S].rearrange("p (t b) -> p b t", b=4),
                                axis=AX.X, op=OP.max)

                # bp decode + mask (independent of LPP; also serves as filler)
                # bp row k = bpw_{k+1} at PHBW cols 8k+20 .. 8k+24
                bp_src = PHBW[:, 16:16 + 8 * K].rearrange("p (k c) -> p k c", c=8)[:, :, 4:8]
                xs2_kb = XS2[:, 0:4 * K].rearrange("p (k b) -> p k b", b=4)
                scr_kb = SCR[:].rearrange("p (k b) -> p k b", b=4)
                mkf_kb = MKF[:, 4:4 * K + 4].rearrange("p (k b) -> p k b", b=4)
                v.tensor_scalar(out=xs2_kb, in0=bp_src,
                                scalar1=-1.0, scalar2=31.0, op0=OP.mult, op1=OP.add)

                # pointer = argmax_i(LP[b,i] + trans[i,END]); one-time tail.
                # Independent ops (incl. the relocated bp-mask bulk op) serve
                # as spacers between each end-write -> start-read pair --
                # explicit drains cost ~2us each on HW.
                v.transpose(out=TLP[:], in_=LPP[:])
                v.tensor_tensor(out=scr_kb, in0=xs2_kb, in1=mkf_kb, op=OP.mult)
                v.tensor_tensor(out=CAND[:], in0=TLP[:], in1=TEND[:], op=OP.add)
                v.tensor_reduce(out=MX[:], in_=CAND[:], axis=AX.X, op=OP.max)
                v.tensor_copy(out=PR[:], in_=CST[:, 0:32])
                v.tensor_tensor(out=EQC[:], in0=CAND[:],
                                in1=MX[:].broadcast_to([128, 32]), op=OP.is_equal)
                v.tensor_tensor(out=SC[:], in0=EQC[:], in1=CST[:, 32:64], op=OP.mult)
                v.tensor_reduce(out=PW[:], in_=SC[:], axis=AX.X, op=OP.max)
                v.tensor_copy(out=PR[:], in_=CST[:, 0:32])
                v.tensor_scalar(out=P32[:, 0:1], in0=PW[:], scalar1=-1.0,
                                scalar2=31.0, op0=OP.mult, op1=OP.add)
                v.tensor_copy(out=CAND[:], in_=TEND[:])

                # scatter pointer at k == last_pos: bp' = bp + atlast*(ptr - bp)
                v.transpose(out=T32[:], in_=P32[:])
                v.stream_shuffle(out=PR[:], in_=T32[:], mask=[0] * 32)
                v.tensor_copy(out=CAND[:], in_=TEND[:])
                pr_b = PR[:, 0:4].unsqueeze(1).broadcast_to([128, K, 4])
                bp_v = SCR[:].rearrange("p (k b) -> p k b", b=4)
                xs_v = XS[:, 0:4 * K].rearrange("p (k b) -> p k b", b=4)
                xs2_v = XS2[:, 0:4 * K].rearrange("p (k b) -> p k b", b=4)
                alf_v = ALF[:, 0:4 * K].rearrange("p (k b) -> p k b", b=4)
                v.tensor_tensor(out=xs_v, in0=pr_b, in1=bp_v, op=OP.subtract)
                v.tensor_tensor(out=xs2_v, in0=xs_v, in1=alf_v, op=OP.mult)
                v.tensor_tensor(out=xs_v, in0=bp_v, in1=xs2_v, op=OP.add)

                # ---- backward pass ----
                # Bulk-transposed bp rows: one 4-block stream-transpose over
                # XS[:, 4k0 : 4k0+128] fills ring slots for ks {k0, k0+8,
                # k0+16, k0+24} at once (slot(k) = k % 32, 32 cols each).
                # The stt pointer-chase then runs as an ADJACENT chain
                # (151ns/step vs 613ns with per-step transposes); the 8
                # transposes of batch F-32 are emitted right after stt(F+m),
                # when all four ks sharing their slots are consumed.
                v.tensor_copy(out=DEC[:, S - 1:S], in_=P32[:, 0:1])
                tb32_r = TB32[:].rearrange("p (g m c) -> p g m c", g=4, m=8, c=32)

                def bigtr(k0):
                    if sim_compat:
                        # CoreSim can't execute blocked multi-tile transposes;
                        # emit the 4 32x32 block transposes separately
                        for g in range(4):
                            v.transpose(
                                out=TB32[:, 32 * (k0 % 32 + 8 * g):32 * (k0 % 32 + 8 * g) + 32],
                                in_=XS[:, 4 * k0 + 32 * g:4 * k0 + 32 * g + 32])
                    else:
                        v.transpose(out=tb32_r[:, :, k0 % 32, :],
                                    in_=XS[:, 4 * k0:4 * k0 + 128].rearrange(
                                        "p (g c) -> p g c", c=32))

                for m in range(8):
                    bigtr(480 + m)
                # stt pointer-chase with one independent spacer op between
                # consecutive stts: fully adjacent stts race (the scalar
                # read is NOT interlocked against the accumulate retire).
                # 8 of every 32 spacer slots are the useful batch
                # transposes; the rest are cheap dummy copies.
                for k in range(S - 2, -1, -1):
                    v.scalar_tensor_tensor(out=EQC[:], in0=CST[:, 0:32],
                                           scalar=DEC[:, k + 1:k + 2],
                                           in1=TB32[:, 32 * (k % 32):32 * (k % 32) + 32],
                                           op0=OP.is_equal, op1=OP.mult,
                                           accum_out=DEC[:, k:k + 1])
                    F = (k // 32) * 32
                    if k % 32 < 8 and F >= 32:
                        bigtr(F - 32 + (k % 32))
                    else:
                        v.tensor_copy(out=CAND[:], in_=CST[:, 0:32])

                v.tensor_copy(out=CAND[:], in_=TEND[:])
                v.tensor_copy(out=DECI[:], in_=DEC[:])

            @block.vector
            def _(v):
                v.wait_ge(dma_sem, 80)
                # reps as a HARDWARE loop: program size independent of reps
                # (body is idempotent -- all state re-initialized per pass)
                with v.Fori(0, reps):
                    emit_body(v)
                v.drain().then_inc(done_sem, 1)

    return nc


def pack_inputs(feats, transitions, mask, S):
    """Host-side layout packing (pure data movement, no arithmetic beyond
    dtype conversion of the 0/1 mask)."""
    trans = np.ascontiguousarray(np.asarray(transitions, np.float32))
    ttrep = np.ascontiguousarray(np.tile(trans.T, (4, 1)))  # [128, 32]
    iota = np.arange(32, dtype=np.float32)
    cst = np.ascontiguousarray(
        np.tile(np.concatenate([iota, 31.0 - iota])[None, :], (128, 1)))
    # materialized descending iota over the (br, i) free layout: plain 2-dim
    # operand for the backpointer-encode multiply
    ct2 = np.ascontiguousarray(
        np.tile(np.tile(31.0 - iota, 4)[None, :], (128, 1)).astype(np.float32))
    in_maps = []
    bc = 16
    for c in range(NCORES):
        f = np.asarray(feats[bc * c:bc * c + bc], np.float32)  # [16, S, 32]
        ft = np.ascontiguousarray(
            f.reshape(4, 4, S, T).transpose(0, 3, 2, 1).reshape(128, 4 * S))
        m = np.asarray(mask[bc * c:bc * c + bc]).astype(np.float32)  # [16, S]
        mk = np.broadcast_to(
            m.reshape(4, 1, 4, S).transpose(0, 1, 3, 2), (4, 32, S, 4))
        mk = mk.reshape(128, 4 * S)
        mkp = np.zeros((128, 4 * S + 4), np.float32)
        mkp[:, :4 * S] = mk
        in_maps.append({"ft": ft, "mkf": mkp, "tt": ttrep, "cst": cst, "ct2": ct2})
    return in_maps


def unpack_outputs(results, S):
    out = np.empty((128, S), np.int32)
    bc = 16
    for c in range(NCORES):
        d = np.asarray(results[c]["dec"]).reshape(4, 32, S)
        out[bc * c:bc * c + bc] = d[:, 0:4, :].reshape(16, S)
    return out


_NC_CACHE = {}


def kernel(feats, transitions, mask):
    B, S, Tin = feats.shape
    assert (B, Tin) == (128, 32)
    if S not in _NC_CACHE:
        _NC_CACHE[S] = build_nc(S)
    nc = _NC_CACHE[S]
    in_maps = pack_inputs(feats, transitions, mask, S)
    res = run_bass_kernel_spmd(nc, in_maps, list(range(NCORES)))
    return unpack_outputs(res.results, S)
